# revision 14
# baseline (speedup 1.0000x reference)
"""GCN + 2-step APPNP propagation on 8 Trainium2 NeuronCores.

Reference computation (N=16384, NFEAT=500, HIDDEN=32, NCLASS=3, alpha=0.25):
    h   = relu(input @ W1)
    l0  = h @ W2
    deg = adj.sum(axis=1);  d = (1 - alpha) / max(deg, 1e-12)
    l1  = d * (adj @ l0) + alpha * l0
    l2  = d * (adj @ l1) + alpha * l0
    out = log_softmax(l2, axis=1)

Distribution: 1D row partition of the graph; core r owns rows
r*2048..(r+1)*2048.  The dominant cost is streaming adj twice.

Layout: TensorE contracts over the partition axis, so adj @ L needs adj's
column index on partitions; each core gets T_r = adj[rows_r, :].T,
host-permuted into contiguous [block, 128, 8*2048] DMA tiles (16 KiB
per-partition lines keep HWDGE descriptor generation off the critical
path) and quantized to fp8-e4m3 (4x less HBM traffic than fp32; output
error ~1e-4 relative since the propagated term is small next to the fp32
alpha*l0 term and quantization noise averages over 16k-term dots).
Chunk-pairs of L are the stationary operand via fp8 DoubleRow (halves PE
streaming time); T_r is the moving operand.

deg rides along pass 1 as a ones-column of L0.  Between passes the tiny
per-core logits are AllGathered through a DRAM bounce.  The bounce DMAs
are threaded INTO the sync-queue FIFO at fixed positions (explicit dep
edges): the saturated HWDGE ring otherwise starves other queues and a
32 KiB bounce takes ~40 us mid-stream.  Four stream blocks sit between
the bounce write and the gather-back so the ~25 us collective latency is
hidden under useful streaming.  Output leaves chunk-major [128, 16, 3]
and is un-permuted on the host.
"""

import os

import numpy as np
import ml_dtypes

import concourse.bass as bass
import concourse.mybir as mybir
import concourse.bacc as bacc
import concourse.tile as tile
from concourse import bass_utils
from concourse.bass import _add_dep_helper

N = 16384
NFEAT = 500
HIDDEN = 32
NCLASS = 3
ALPHA = 0.25
NCORES = 8
ROWS = N // NCORES        # 2048 rows owned per core
P = 128                   # SBUF partitions
CHUNKS = N // P           # 128 global row-chunks
LCH = ROWS // P           # 16 local row-chunks
NB = 8                    # row-chunks per adj DMA block
NBLK = N // (NB * P)      # 16 stream blocks per pass
ISL = 512                 # moving-operand free-dim per matmul
NISL = ROWS // ISL        # 4 output column slices
TT_BUFS = 9               # adj stream prefetch depth (x2 MiB)
LPAD = 16                 # L-chunk stride (DoubleRow needs step%16==0)
K1 = 2                    # stream blocks ahead of the bounce write
K3 = 4                    # stream blocks between bounce write and gather

F32 = mybir.dt.float32
BF16 = mybir.dt.bfloat16
ADT = mybir.dt.float8e4
ADT_NP = ml_dtypes.float8_e4m3
BF16_NP = ml_dtypes.bfloat16
AF = mybir.ActivationFunctionType
ALU = mybir.AluOpType
AX = mybir.AxisListType
DR = mybir.MatmulPerfMode.DoubleRow

_COMPILED = None
LAST_EXEC_TIME_NS = None
LAST_RESULTS = None


def _build():
    nc = bacc.Bacc("TRN2", target_bir_lowering=False, debug=False,
                   num_devices=NCORES)

    t_d = nc.dram_tensor("t", [NBLK, P, NB * ROWS], ADT,
                         kind="ExternalInput").ap()
    xt_d = nc.dram_tensor("xt", [NFEAT, ROWS], BF16, kind="ExternalInput").ap()
    w1_d = nc.dram_tensor("w1", [NFEAT, HIDDEN], BF16,
                          kind="ExternalInput").ap()
    w2_d = nc.dram_tensor("w2", [HIDDEN, NCLASS], F32,
                          kind="ExternalInput").ap()
    eye_d = nc.dram_tensor("eye", [4, 4], F32, kind="ExternalInput").ap()
    out_d = nc.dram_tensor("out", [P, LCH * NCLASS], F32,
                           kind="ExternalOutput").ap()

    rg = [list(range(NCORES))]

    with tile.TileContext(nc) as tc:
        with (
            tc.tile_pool(name="const", bufs=1) as const,
            tc.tile_pool(name="persist", bufs=1) as persist,
            tc.tile_pool(name="ttp", bufs=TT_BUFS) as ttp,
            tc.tile_pool(name="dram", bufs=1, space="DRAM") as dram,
        ):
            eye_sb = const.tile([4, 4], F32)
            nc.gpsimd.dma_start(eye_sb[:], eye_d[:])
            w2_sb = const.tile([HIDDEN, NCLASS], F32)
            nc.gpsimd.dma_start(w2_sb[:], w2_d[:])

            # live across the whole kernel
            alpha_l0 = persist.tile([P, LCH, NCLASS], F32)    # 0.25*l0, local
            d_all = persist.tile([P, LCH], F32)               # 0.75/deg, local
            l0_rhs = persist.tile([P, CHUNKS, LPAD], ADT)     # [l0 | 1] chunks
            l1_rhs = persist.tile([P, CHUNKS, LPAD], ADT)     # l1 chunks
            l0c = persist.tile([P, LCH, LPAD], ADT)           # local AG payload
            l1c = persist.tile([P, LCH, LPAD], ADT)           # local AG payload
            out_sb = persist.tile([P, LCH, NCLASS], F32)
            y1T = persist.tile([4, ROWS], F32)
            y2T = persist.tile([NCLASS, ROWS], F32)

            cc1_in = dram.tile([ROWS * LPAD], ADT)
            cc1_out = dram.tile([N * LPAD], ADT)
            cc2_in = dram.tile([ROWS * LPAD], ADT)
            cc2_out = dram.tile([N * LPAD], ADT)

            # adj stream helper: one contiguous 2 MiB block DMA
            def stream_block(idx):
                tt = ttp.tile([P, NB * ROWS], ADT, name="tt", tag="tt")
                return tt, nc.sync.dma_start(tt[:], t_d[idx])

            # ---- stage 1: local l0 = relu(x @ W1) @ W2 (transposed forms) --
            ksz = [P, P, P, NFEAT - 3 * P]  # 500 = 128*3 + 116
            with (
                tc.tile_pool(name="s1sb", bufs=1) as s1sb,
                tc.tile_pool(name="hpsp", bufs=1, space="PSUM") as hpsp,
                tc.tile_pool(name="l0psp", bufs=1, space="PSUM") as l0psp,
            ):
                w1c, xtc = [], []
                for k in range(4):
                    w = s1sb.tile([ksz[k], HIDDEN], BF16, name=f"w1c{k}")
                    nc.sync.dma_start(w[:], w1_d[k * P:k * P + ksz[k], :])
                    w1c.append(w)
                for k in range(4):
                    x = s1sb.tile([ksz[k], ROWS], BF16, name=f"xtc{k}")
                    nc.sync.dma_start(x[:], xt_d[k * P:k * P + ksz[k], :])
                    xtc.append(x)

                # pass-1 stream head starts right after the stage-1 inputs
                p1 = [stream_block(b) for b in range(K1)]

                hps = [hpsp.tile([HIDDEN, ISL], F32, name=f"hps{i}",
                                 tag=f"hps{i}") for i in range(NISL)]
                # k outer so the last xt chunk's arrival is the long pole
                for k in range(4):
                    for i in range(NISL):
                        nc.tensor.matmul(
                            hps[i][:], w1c[k][:],
                            xtc[k][:, i * ISL:(i + 1) * ISL],
                            start=(k == 0), stop=(k == 3))
                hT = s1sb.tile([HIDDEN, ROWS], F32)
                for i in range(NISL):
                    nc.scalar.activation(hT[:, i * ISL:(i + 1) * ISL],
                                         hps[i][:], AF.Relu)

                l0ps = l0psp.tile([P, LCH, NCLASS], F32)
                for n in range(LCH):
                    nc.tensor.matmul(l0ps[:, n, :], hT[:, n * P:(n + 1) * P],
                                     w2_sb[:], start=True, stop=True)
                nc.vector.tensor_scalar_mul(alpha_l0[:], l0ps[:], ALPHA)
                nc.scalar.activation(l0c[:, :, 0:NCLASS], l0ps[:], AF.Copy)
                nc.vector.memset(l0c[:, :, NCLASS], 1.0)
                nc.vector.memset(l0c[:, :, NCLASS + 1:LPAD], 0.0)

            # ---- all-gather l0, threaded into the sync FIFO ---------------
            cc1_w = nc.sync.dma_start(
                cc1_in[:].rearrange("(p f) -> p f", p=P),
                l0c[:].rearrange("p n f -> p (n f)"))
            _add_dep_helper(cc1_w.ins, p1[K1 - 1][1].ins, reason="fifo order")
            for b in range(K1, K1 + K3):
                blk = stream_block(b)
                _add_dep_helper(blk[1].ins, cc1_w.ins, reason="fifo order")
                p1.append(blk)
            nc.gpsimd.collective_compute(
                "AllGather", ALU.bypass, replica_groups=rg,
                ins=[cc1_in.opt()], outs=[cc1_out.opt()])
            g1 = nc.sync.dma_start(
                l0_rhs[:].rearrange("p c f -> p (c f)")
                .rearrange("p (k f) -> p k f", k=NCORES),
                cc1_out[:].rearrange("(k p f) -> p k f", k=NCORES, p=P))
            _add_dep_helper(g1.ins, p1[K1 + K3 - 1][1].ins, reason="fifo order")
            for b in range(K1 + K3, NBLK):
                blk = stream_block(b)
                _add_dep_helper(blk[1].ins, g1.ins, reason="fifo order")
                p1.append(blk)

            # ---- propagation pass 1: y1 = adj @ [l0 | 1] ------------------
            with tc.tile_pool(name="y1ps", bufs=1, space="PSUM") as y1psp:
                y1ps = [y1psp.tile([4, ISL], F32, name=f"y1ps{i}",
                                   tag=f"y1ps{i}") for i in range(NISL)]
                for b in range(NBLK):
                    tt3 = p1[b][0][:].rearrange("p (s f) -> p s f", s=NB)
                    for s2 in range(NB // 2):
                        jc = b * NB + 2 * s2
                        for i in range(NISL):
                            nc.tensor.matmul(
                                y1ps[i][:], l0_rhs[:, jc:jc + 2, 0:4],
                                tt3[:, 2 * s2:2 * s2 + 2,
                                    i * ISL:(i + 1) * ISL],
                                start=(jc == 0), stop=(jc == CHUNKS - 2),
                                perf_mode=DR)
                for i in range(NISL):
                    nc.scalar.activation(y1T[:, i * ISL:(i + 1) * ISL],
                                         y1ps[i][:], AF.Copy)

            # ---- iteration update: l1 = d*y1 + alpha*l0 -------------------
            with (
                tc.tile_pool(name="upd", bufs=1) as upd,
                tc.tile_pool(name="updps", bufs=1, space="PSUM") as updps,
            ):
                ytp = updps.tile([P, LCH, 4], F32)
                for n in range(LCH):
                    nc.tensor.transpose(ytp[:, n, :],
                                        y1T[:, n * P:(n + 1) * P], eye_sb[:])
                dmx = upd.tile([P, LCH], F32)
                nc.vector.tensor_scalar_max(dmx[:], ytp[:, :, 3], 1e-12)
                rec = upd.tile([P, LCH], F32)
                nc.vector.reciprocal(rec[:], dmx[:])
                nc.vector.tensor_scalar_mul(d_all[:], rec[:], 1.0 - ALPHA)
                ty = upd.tile([P, LCH, NCLASS], F32)
                nc.vector.tensor_mul(ty[:], ytp[:, :, 0:NCLASS],
                                     d_all[:].broadcast_to([P, LCH, NCLASS]))
                nc.vector.tensor_add(l1c[:, :, 0:NCLASS], ty[:], alpha_l0[:])
                nc.vector.memset(l1c[:, :, NCLASS:LPAD], 0.0)

            # ---- all-gather l1, threaded into the pass-2 stream -----------
            p2 = [stream_block(b) for b in range(K1)]
            cc2_w = nc.sync.dma_start(
                cc2_in[:].rearrange("(p f) -> p f", p=P),
                l1c[:].rearrange("p n f -> p (n f)"))
            _add_dep_helper(cc2_w.ins, p2[K1 - 1][1].ins, reason="fifo order")
            for b in range(K1, K1 + K3):
                blk = stream_block(b)
                _add_dep_helper(blk[1].ins, cc2_w.ins, reason="fifo order")
                p2.append(blk)
            nc.gpsimd.collective_compute(
                "AllGather", ALU.bypass, replica_groups=rg,
                ins=[cc2_in.opt()], outs=[cc2_out.opt()])
            g2 = nc.sync.dma_start(
                l1_rhs[:].rearrange("p c f -> p (c f)")
                .rearrange("p (k f) -> p k f", k=NCORES),
                cc2_out[:].rearrange("(k p f) -> p k f", k=NCORES, p=P))
            _add_dep_helper(g2.ins, p2[K1 + K3 - 1][1].ins, reason="fifo order")
            for b in range(K1 + K3, NBLK):
                blk = stream_block(b)
                _add_dep_helper(blk[1].ins, g2.ins, reason="fifo order")
                p2.append(blk)

            # ---- propagation pass 2: y2 = adj @ l1 ------------------------
            with tc.tile_pool(name="y2ps", bufs=1, space="PSUM") as y2psp:
                y2ps = [y2psp.tile([NCLASS, ISL], F32, name=f"y2ps{i}",
                                   tag=f"y2ps{i}") for i in range(NISL)]
                for b in range(NBLK):
                    tt3 = p2[b][0][:].rearrange("p (s f) -> p s f", s=NB)
                    for s2 in range(NB // 2):
                        jc = b * NB + 2 * s2
                        for i in range(NISL):
                            nc.tensor.matmul(
                                y2ps[i][:], l1_rhs[:, jc:jc + 2, 0:NCLASS],
                                tt3[:, 2 * s2:2 * s2 + 2,
                                    i * ISL:(i + 1) * ISL],
                                start=(jc == 0), stop=(jc == CHUNKS - 2),
                                perf_mode=DR)
                for i in range(NISL):
                    nc.scalar.activation(y2T[:, i * ISL:(i + 1) * ISL],
                                         y2ps[i][:], AF.Copy)

            # ---- final update + log_softmax -------------------------------
            with (
                tc.tile_pool(name="fin", bufs=1) as fin,
                tc.tile_pool(name="finps", bufs=1, space="PSUM") as finps,
            ):
                y2tp = finps.tile([P, LCH, NCLASS], F32)
                for n in range(LCH):
                    nc.tensor.transpose(y2tp[:, n, :],
                                        y2T[:, n * P:(n + 1) * P],
                                        eye_sb[0:NCLASS, 0:NCLASS])
                lg = fin.tile([P, LCH, NCLASS], F32)
                nc.vector.tensor_mul(lg[:], y2tp[:],
                                     d_all[:].broadcast_to([P, LCH, NCLASS]))
                nc.vector.tensor_add(lg[:], lg[:], alpha_l0[:])
                negm = fin.tile([P, LCH], F32)
                nc.vector.tensor_reduce(negm[:], lg[:], axis=AX.X, op=ALU.max,
                                        negate=True)
                lgm = fin.tile([P, LCH, NCLASS], F32)
                nc.vector.tensor_add(lgm[:], lg[:],
                                     negm[:].broadcast_to([P, LCH, NCLASS]))
                ex = fin.tile([P, LCH, NCLASS], F32)
                nc.scalar.activation(ex[:], lgm[:], AF.Exp)
                sm = fin.tile([P, LCH], F32)
                nc.vector.tensor_reduce(sm[:], ex[:], axis=AX.X, op=ALU.add)
                rs = fin.tile([P, LCH], F32)
                nc.vector.reciprocal(rs[:], sm[:])
                nls = fin.tile([P, LCH], F32)
                nc.scalar.activation(nls[:], rs[:], AF.Ln)
                nc.vector.tensor_add(out_sb[:], lgm[:],
                                     nls[:].broadcast_to([P, LCH, NCLASS]))

            nc.gpsimd.dma_start(out_d[:],
                                out_sb[:].rearrange("p n f -> p (n f)"))

    nc.compile()
    return nc


def kernel(input, adj, W1, W2):
    """Full inputs in, full [N, NCLASS] float32 log-softmax out."""
    global _COMPILED, LAST_EXEC_TIME_NS, LAST_RESULTS
    if _COMPILED is None:
        _COMPILED = _build()
    nc = _COMPILED

    input = np.asarray(input, dtype=np.float32)
    adj = np.asarray(adj, dtype=np.float32)
    W1 = np.asarray(W1, dtype=np.float32)
    W2 = np.asarray(W2, dtype=np.float32)

    adj_q = adj.astype(ADT_NP)
    xt = np.ascontiguousarray(input.T).astype(BF16_NP)
    w1_q = W1.astype(BF16_NP)
    eye = np.eye(4, dtype=np.float32)

    in_maps = []
    for r in range(NCORES):
        t_r = np.ascontiguousarray(
            adj_q[r * ROWS:(r + 1) * ROWS, :].T
            .reshape(NBLK, NB, P, ROWS)
            .transpose(0, 2, 1, 3)
            .reshape(NBLK, P, NB * ROWS))
        in_maps.append({
            "t": t_r,
            "xt": np.ascontiguousarray(xt[:, r * ROWS:(r + 1) * ROWS]),
            "w1": w1_q,
            "w2": W2,
            "eye": eye,
        })

    res = bass_utils.run_bass_kernel_spmd(
        nc, in_maps, core_ids=list(range(NCORES)),
        trace=bool(os.environ.get("GNN_TRACE")))
    LAST_EXEC_TIME_NS = res.exec_time_ns
    LAST_RESULTS = res

    out = np.empty((N, NCLASS), dtype=np.float32)
    for r in range(NCORES):
        blk = res.results[r]["out"].reshape(P, LCH, NCLASS)
        out[r * ROWS:(r + 1) * ROWS] = (
            blk.transpose(1, 0, 2).reshape(ROWS, NCLASS))
    return out
